# revision 44
# baseline (speedup 1.0000x reference)
"""DIST loss (hard CE + inter/intra Pearson distillation) on 8 Trainium2 cores.

Strategy: data-parallel over the batch dim (4096 rows -> 512 rows/core).
Each core streams its [512, 32000] f32 shard once from HBM in 3072-col
slabs -- z_s on the SP HWDGE ring, z_t on the GPSIMD SWDGE ring so two
descriptor generators feed the 16 SDMA engines. ScalarE computes bf16
exp() slabs (f32 row-sum partials free via the activation accumulator).
Per 128-row block:
  - products + fused row-sums, slab-granular, split across engines so
    ScalarE and VectorE both run ~95us/block: p11 = Square(es) on ScalarE
    (accumulator -> U11) for 3 of 4 slabs, p22/p12 (and the rest of p11)
    as scalar_tensor_tensor on VectorE (accum_out -> U22/U12).
  - per-column weighted sums via TensorE: stat k's per-row weight lives in
    column k of an otherwise-zero [128, 32] stationary tile (32 wide so
    every PSUM partition of the output group is initialized), so 5
    accumulating matmuls (es, et, p11, p22, p12 as moving operands) build
    [32, n] PSUM blocks at base partitions 0/32/64 of one bank.
  - VectorE accumulates each chunk's PSUM block into a bf16 SBUF
    accumulator across the 4 row blocks; the last block streams the
    accumulated column stats (2.1 MB) out per-chunk on the by-then-idle
    SP ring, overlapping the compute tail.
The host sums the partial column stats over cores and finishes the
O(B + C) scalar math (Pearson means, label gather, log) in float64.
"""
import sys
import types
import numpy as np

sys.path.insert(0, "/opt/trn_rl_repo")

B, C = 4096, 32000
N_CORES = 8
R = B // N_CORES          # 512 rows per core
P = 128                   # partitions
NBLK = R // P             # 4 row blocks per core
SLAB = 3072               # DMA/exp granularity (12KB HBM descriptors)
NSLB = 11                 # 10 full slabs + 1280-col tail
SLABS = [(i * SLAB, SLAB) for i in range(NSLB - 1)] + [
    ((NSLB - 1) * SLAB, C - (NSLB - 1) * SLAB)
]
CHUNK = 1536              # PE/PSUM granularity (3 sub-matmuls @ part 0/32/64)
NCH = 21                  # 20 full chunks + 1280-col tail
CHUNKS = [(i * CHUNK, CHUNK) for i in range(NCH - 1)] + [
    ((NCH - 1) * CHUNK, C - (NCH - 1) * CHUNK)
]
EPS = 1e-8

_built = None


def _install_ntff_shim():
    # antenv.axon_hooks is absent in this image; register the ctypes NTFF
    # hook so run_bass_kernel_spmd(trace=True) can profile under axon.
    try:
        import antenv
        import trn_agent_boot.trn_boot as tb
        if "antenv.axon_hooks" in sys.modules:
            return
        hook = tb._ntff_profile_via_ctypes("/opt/axon/libaxon_pjrt.so")
        mod = types.ModuleType("antenv.axon_hooks")
        mod.get_axon_ntff_profile_hook = lambda: hook
        mod.set_axon_ntff_profile_hook = lambda h: None
        antenv.axon_hooks = mod
        sys.modules["antenv.axon_hooks"] = mod
    except Exception:
        pass


def _sub_slices(cw):
    subs = []
    o = 0
    while o < cw:
        n = min(512, cw - o)
        subs.append((o, n))
        o += n
    return subs


def _build():
    from contextlib import ExitStack
    import concourse.bacc as bacc
    import concourse.tile as tile
    from concourse import mybir

    f32 = mybir.dt.float32
    bf16 = mybir.dt.bfloat16
    Exp = mybir.ActivationFunctionType.Exp
    Square = mybir.ActivationFunctionType.Square
    ADD = mybir.AluOpType.add
    MUL = mybir.AluOpType.mult
    AXF = mybir.AxisListType.X

    nc = bacc.Bacc("TRN2", target_bir_lowering=False, debug=False)
    zs_d = nc.dram_tensor("z_s", [R, C], f32, kind="ExternalInput")
    zt_d = nc.dram_tensor("z_t", [R, C], f32, kind="ExternalInput")
    # [psum partition, chunk*512]: rows 32s..32s+4 hold stats 0..4 of
    # sub-matmul s (block-summed on device); the host skips filler rows.
    col_d = nc.dram_tensor("colstats", [96, NCH * 512], bf16,
                           kind="ExternalOutput")
    row_d = nc.dram_tensor("rowstats", [R, 8], f32, kind="ExternalOutput")

    with tile.TileContext(nc) as tc, ExitStack() as ctx:
        zin = ctx.enter_context(tc.tile_pool(name="zin", bufs=3))
        esp = ctx.enter_context(tc.tile_pool(name="esp", bufs=1))
        etp = ctx.enter_context(tc.tile_pool(name="etp", bufs=1))
        prodp = ctx.enter_context(tc.tile_pool(name="prodp", bufs=4))
        accp = ctx.enter_context(tc.tile_pool(name="accp", bufs=1))
        small = ctx.enter_context(tc.tile_pool(name="small", bufs=2))
        psump = ctx.enter_context(tc.tile_pool(name="psum", bufs=8, space="PSUM"))

        rsp = ctx.enter_context(tc.tile_pool(name="rsp", bufs=NBLK))
        acc = accp.tile([96, NCH * 512], bf16, tag="acc")
        nc.gpsimd.memset(acc[:], 0.0)
        rs_tiles = []

        blk = [{} for _ in range(NBLK)]

        def emit_A(b, si):
            r0 = b * P
            c0, cw = SLABS[si]
            st = blk[b]
            if si == 0:
                st["zsp"] = small.tile([P, NSLB], f32, tag="zsp", name="zsp")
                st["ztp"] = small.tile([P, NSLB], f32, tag="ztp", name="ztp")
                st["es"] = []
                st["et"] = []
            zs = zin.tile([P, cw], f32, tag="zin")
            nc.sync.dma_start(zs[:], zs_d[r0:r0 + P, c0:c0 + cw])
            es = esp.tile([P, cw], bf16, tag=f"es{si}")
            nc.scalar.activation(es[:], zs[:], Exp,
                                 accum_out=st["zsp"][:, si:si + 1])
            zt = zin.tile([P, cw], f32, tag="zin")
            nc.gpsimd.dma_start(zt[:], zt_d[r0:r0 + P, c0:c0 + cw])
            et = etp.tile([P, cw], bf16, tag=f"et{si}")
            nc.scalar.activation(et[:], zt[:], Exp,
                                 accum_out=st["ztp"][:, si:si + 1])
            st["es"].append(es)
            st["et"].append(et)

        def emit_W(b):
            st = blk[b]
            rs = rsp.tile([P, 8], f32, tag="rs")
            rs_tiles.append(rs)
            st["rs"] = rs
            st["u11p"] = small.tile([P, NSLB], f32, tag="u11p", name="u11p")
            st["u22p"] = small.tile([P, NSLB], f32, tag="u22p", name="u22p")
            st["u12p"] = small.tile([P, NSLB], f32, tag="u12p", name="u12p")
            nc.vector.tensor_reduce(rs[:, 0:1], st["zsp"][:, 0:NSLB],
                                    axis=AXF, op=ADD)
            nc.vector.tensor_reduce(rs[:, 1:2], st["ztp"][:, 0:NSLB],
                                    axis=AXF, op=ADD)
            w1 = small.tile([P, 1], f32, tag="w1")
            nc.vector.reciprocal(w1[:], rs[:, 0:1])
            w2 = small.tile([P, 1], f32, tag="w2")
            nc.vector.reciprocal(w2[:], rs[:, 1:2])
            # stat k's per-row weight lives in column k of an otherwise-zero
            # [P, 32] stationary tile (32 wide so every PSUM partition of the
            # matmul output group is initialized)
            W_tiles = []
            for k in range(5):
                Wk = small.tile([P, 32], bf16, tag=f"W{k}")
                nc.vector.memset(Wk[:], 0.0)
                W_tiles.append(Wk)
            nc.vector.tensor_copy(W_tiles[0][:, 0:1], w1[:])
            nc.vector.tensor_copy(W_tiles[1][:, 1:2], w2[:])
            nc.vector.tensor_mul(W_tiles[2][:, 2:3], w1[:], w1[:])
            nc.vector.tensor_mul(W_tiles[3][:, 3:4], w2[:], w2[:])
            nc.vector.tensor_mul(W_tiles[4][:, 4:5], w1[:], w2[:])
            st["W"] = W_tiles

        def emit_B(b, si):
            st = blk[b]
            s0, sw = SLABS[si]
            es_s = st["es"][si]
            et_s = st["et"][si]
            W_tiles = st["W"]
            # slab-granular product + fused row-sum, split so ScalarE and
            # VectorE run ~equally loaded
            p11 = prodp.tile([P, sw], bf16, tag="prods")
            if b == NBLK - 1 or si % 2 == 0:
                nc.scalar.activation(p11[:], es_s[:], Square,
                                     accum_out=st["u11p"][:, si:si + 1])
            else:
                nc.vector.scalar_tensor_tensor(
                    p11[:], es_s[:], 1.0, es_s[:], MUL, MUL,
                    accum_out=st["u11p"][:, si:si + 1])
            p22 = prodp.tile([P, sw], bf16, tag="prods")
            if b == NBLK - 1:
                nc.scalar.activation(p22[:], et_s[:], Square,
                                     accum_out=st["u22p"][:, si:si + 1])
            else:
                nc.vector.scalar_tensor_tensor(
                    p22[:], et_s[:], 1.0, et_s[:], MUL, MUL,
                    accum_out=st["u22p"][:, si:si + 1])
            p12 = prodp.tile([P, sw], bf16, tag="prods")
            nc.vector.scalar_tensor_tensor(
                p12[:], es_s[:], 1.0, et_s[:], MUL, MUL,
                accum_out=st["u12p"][:, si:si + 1])
            for ci in range(si * 2, min(si * 2 + 2, NCH)):
                c0, cw = CHUNKS[ci]
                soff = c0 - s0
                rhs_list = [es_s[:, soff:soff + cw],
                            et_s[:, soff:soff + cw],
                            p11[:, soff:soff + cw],
                            p22[:, soff:soff + cw],
                            p12[:, soff:soff + cw]]
                ps = psump.tile([96, 512], f32, tag="ps")
                for s_i, (o, n) in enumerate(_sub_slices(cw)):
                    for k in range(5):
                        nc.tensor.matmul(ps[32 * s_i:32 * s_i + 32, 0:n],
                                         W_tiles[k][:, 0:32],
                                         rhs_list[k][:, o:o + n],
                                         start=(k == 0), stop=(k == 4))
                a0 = ci * 512
                if cw == CHUNK:
                    aslc = acc[:, a0:a0 + 512]
                    nc.vector.tensor_add(aslc, aslc, ps[:])
                else:
                    # tail chunk: third 32-partition group only has 256 cols
                    ah = acc[0:64, a0:a0 + 512]
                    nc.vector.tensor_add(ah, ah, ps[0:64, 0:512])
                    at = acc[64:96, a0:a0 + 256]
                    nc.vector.tensor_add(at, at, ps[64:96, 0:256])
                if b == NBLK - 1:
                    nc.sync.dma_start(col_d[:, a0:a0 + 512],
                                      acc[:, a0:a0 + 512])

        def emit_U(b):
            st = blk[b]
            rs = st["rs"]
            nc.vector.tensor_reduce(rs[:, 2:3], st["u11p"][:, 0:NSLB],
                                    axis=AXF, op=ADD)
            nc.vector.tensor_reduce(rs[:, 3:4], st["u22p"][:, 0:NSLB],
                                    axis=AXF, op=ADD)
            nc.vector.tensor_reduce(rs[:, 4:5], st["u12p"][:, 0:NSLB],
                                    axis=AXF, op=ADD)

        # Interleaved emission: block b's phase B alternates slab-by-slab
        # with block b+1's phase A, so exps interleave with squares in the
        # in-order ACT queue and the input DMA never starves behind a
        # block's worth of product work.
        for si in range(NSLB):
            emit_A(0, si)
        for b in range(NBLK):
            emit_W(b)
            for si in range(NSLB):
                emit_B(b, si)
                if b + 1 < NBLK:
                    emit_A(b + 1, si)
            emit_U(b)

        for b in range(NBLK):
            nc.sync.dma_start(row_d[b * P:(b + 1) * P, 0:5], rs_tiles[b][:, 0:5])

    nc.compile()
    return nc


def _get_built():
    global _built
    if _built is None:
        _install_ntff_shim()
        _built = _build()
    return _built


def _unpack_col(colstats):
    """colstats [96, NCH*512] bf16 (already block-summed on device) ->
    [5, C] float64 column stats."""
    acc = colstats.astype(np.float64)
    col = np.zeros((5, C), np.float64)
    for ci, (c0, cw) in enumerate(CHUNKS):
        for s, (o, n) in enumerate(_sub_slices(cw)):
            col[:, c0 + o:c0 + o + n] += acc[32 * s:32 * s + 5,
                                             ci * 512:ci * 512 + n]
    return col


def run_sharded(z_s, z_t, trace=False, tmpdir=None):
    """Run the device program; returns (colstats_sum [5, C] f64,
    rowstats [B, 5] f64, BassKernelResults)."""
    from concourse.bass_utils import run_bass_kernel_spmd

    nc = _get_built()
    z_s = np.ascontiguousarray(np.asarray(z_s, dtype=np.float32))
    z_t = np.ascontiguousarray(np.asarray(z_t, dtype=np.float32))
    in_maps = [
        {"z_s": z_s[i * R:(i + 1) * R], "z_t": z_t[i * R:(i + 1) * R]}
        for i in range(N_CORES)
    ]
    res = run_bass_kernel_spmd(nc, in_maps, core_ids=list(range(N_CORES)),
                               trace=trace, tmpdir=tmpdir)
    col = np.zeros((5, C), np.float64)
    rows = []
    for i in range(N_CORES):
        col += _unpack_col(res.results[i]["colstats"])
        rows.append(res.results[i]["rowstats"][:, :5].astype(np.float64))
    return col, np.concatenate(rows, axis=0), res


def kernel(z_s, z_t, labels):
    col, rowstats, _ = run_sharded(z_s, z_t)
    return _finish(np.asarray(z_s), np.asarray(labels), col, rowstats)


def _finish(z_s, labels, col, rowstats):
    Zs, Zt, U11, U22, U12 = rowstats.T
    invC = 1.0 / C
    # inter: Pearson over classes per row (softmax rows have mean 1/C)
    num = U12 / (Zs * Zt) - invC
    vs = U11 / (Zs * Zs) - invC
    vt = U22 / (Zt * Zt) - invC
    corr = num / (np.sqrt(vs) * np.sqrt(vt) + EPS)
    inter = 1.0 - corr.mean()
    # intra: Pearson over samples per column
    S1, S2, S11, S22, S12 = col
    numc = S12 - S1 * S2 / B
    vsc = S11 - S1 * S1 / B
    vtc = S22 - S2 * S2 / B
    corrc = numc / (np.sqrt(vsc) * np.sqrt(vtc) + EPS)
    intra = 1.0 - corrc.mean()
    # hard CE: mean(logsumexp(z_s) - z_s[label])
    lab = np.asarray(labels).astype(np.int64).ravel()
    zl = z_s[np.arange(B), lab].astype(np.float64)
    hard = (np.log(Zs) - zl).mean()
    return np.float32(hard + inter + intra)
